# revision 15
# baseline (speedup 1.0000x reference)
"""Trainium2 Bass kernel for nn_CogAgentDecoderLayer (moe_routing).

DP2 x TP4 across 8 NeuronCores: cores 0-3 batch 0, cores 4-7 batch 1;
within a group, tensor-parallel over heads / intermediate dim with 3
grouped AllReduces. Device activations are feature-major
([features, tokens]); host transposes inputs/outputs and pre-casts
weights to bf16. Tokens are permuted within each 512-block so
vision-expert tokens come first, letting both experts' matmuls run
packed, with a small data-masked overlap strip keeping the program
SPMD-identical across batches.
"""

import numpy as np
import ml_dtypes
from contextlib import ExitStack

H = 2048
NH = 16
HD = 128
CH = 1024
CC = 1024
CHD = 64
ISL = 1408          # per-core intermediate slice (padded to 11*128)
B = 2
L = 2048
LE = 1024
EPS = 1e-6
N_CORES = 8
TP = 4
GROUPS = [[0, 1, 2, 3], [4, 5, 6, 7]]
NBLK = 4
BLK = 512
KT_H = H // 128     # 16
PACKED = True
BF_NP = ml_dtypes.bfloat16

_CACHE = {}


def _tile128(w):
    """[K, M] -> [128, K//128, M] contiguous bf16 (ki = partition)."""
    K, M = w.shape
    return np.ascontiguousarray(
        w.reshape(K // 128, 128, M).transpose(1, 0, 2)
    ).astype(BF_NP)


def host_prep(inputs):
    hs = np.asarray(inputs["hidden_states"], np.float32)
    enc = np.asarray(inputs["encoder_outputs"], np.float32)
    tt = np.asarray(inputs["token_type_ids"])
    pos = np.asarray(inputs["position_ids"])[0]

    vm = np.zeros((B, L), bool)
    vm[:, :-1] = (tt[:, :-1] == 1) & (tt[:, 1:] == 1)

    perms = []
    for b in range(B):
        parts = []
        for blk in range(NBLK):
            idx = np.arange(blk * BLK, (blk + 1) * BLK)
            parts.append(np.concatenate([idx[vm[b, idx]], idx[~vm[b, idx]]]))
        perms.append(np.concatenate(parts))
    nv = [[int(vm[b, blk * BLK:(blk + 1) * BLK].sum()) for blk in range(NBLK)]
          for b in range(B)]
    if PACKED:
        sA = [min(nv[0][k], nv[1][k]) for k in range(NBLK)]
        sB = [max(nv[0][k], nv[1][k]) for k in range(NBLK)]
    else:
        perms = [np.arange(L), np.arange(L)]
        sA = [0] * NBLK
        sB = [BLK] * NBLK
    widths = [sB[k] - sA[k] for k in range(NBLK)]
    offs = np.concatenate([[0], np.cumsum(widths)]).astype(int)
    W = max(int(offs[-1]), 1)

    inv = 1.0 / (10000.0 ** (np.arange(0, HD, 2, dtype=np.float32) / HD))
    freqs = np.outer(pos.astype(np.float32), inv)        # [L, 64]
    Cfull = np.concatenate([np.cos(freqs), np.cos(freqs)], 1).T   # [128, L]
    Sfull = np.concatenate([-np.sin(freqs), np.sin(freqs)], 1).T  # [128, L]
    qscale = 1.0 / np.sqrt(HD)

    lnin = np.asarray(inputs["ln_in_w"], np.float32)
    lnc = np.asarray(inputs["ln_post_cross_w"], np.float32)
    lna = np.asarray(inputs["ln_post_attn_w"], np.float32)

    in_maps = []
    for core in range(N_CORES):
        b, r = core // TP, core % TP
        perm = perms[b]
        m = {}
        m["h0T"] = np.ascontiguousarray(hs[b][perm].T)           # [H, L] f32
        m["eT"] = np.ascontiguousarray(enc[b].T).astype(BF_NP)   # [CH, LE]

        qkv_cols = np.concatenate([
            np.arange(r * 512, (r + 1) * 512),
            H + np.arange(r * 512, (r + 1) * 512),
            2 * H + np.arange(r * 512, (r + 1) * 512)])
        for ex in ("v", "l"):
            wq = np.asarray(inputs[f"{ex}_qkv"], np.float32)
            m[f"wqkv_{ex}"] = _tile128((wq * lnin[:, None])[:, qkv_cols])
            wdn = np.asarray(inputs[f"{ex}_dense"], np.float32)
            m[f"wdense_{ex}"] = _tile128(wdn[r * 512:(r + 1) * 512, :])
            wgt = np.asarray(inputs[f"{ex}_gate"], np.float32)
            wup = np.asarray(inputs[f"{ex}_up"], np.float32)
            wdw = np.asarray(inputs[f"{ex}_down"], np.float32)
            isl = np.arange(r * 1376, (r + 1) * 1376)
            gpad = np.zeros((H, ISL), np.float32)
            gpad[:, :1376] = (wgt * lna[:, None])[:, isl]
            m[f"wg_{ex}"] = _tile128(gpad)
            upad = np.zeros((H, ISL), np.float32)
            upad[:, :1376] = (wup * lna[:, None])[:, isl]
            m[f"wu_{ex}"] = _tile128(upad)
            dpad = np.zeros((ISL, H), np.float32)
            dpad[:1376, :] = wdw[isl, :]
            m[f"wd_{ex}"] = _tile128(dpad)

        wcq = np.asarray(inputs["cq_w"], np.float32)
        m["wcq"] = _tile128((wcq * lnc[:, None] / np.sqrt(CHD))
                            [:, r * 256:(r + 1) * 256])
        wckv = np.asarray(inputs["ckv_w"], np.float32)
        ckv_cols = np.concatenate([np.arange(r * 256, (r + 1) * 256),
                                   CC + np.arange(r * 256, (r + 1) * 256)])
        m["wckv"] = _tile128(wckv[:, ckv_cols])
        wcd = np.asarray(inputs["cdense_w"], np.float32)
        m["wcd"] = _tile128(wcd[r * 256:(r + 1) * 256, :])

        m["ropeq_c"] = (Cfull[:, perm] * qscale).astype(BF_NP).copy()
        m["ropeq_s"] = (Sfull[:, perm] * qscale).astype(BF_NP).copy()
        m["rope_c"] = Cfull[:, perm].astype(BF_NP).copy()
        m["rope_s"] = Sfull[:, perm].astype(BF_NP).copy()

        dm = np.zeros((128, 16, BLK), np.float32)
        pr = perm.reshape(NBLK, 4, 128)
        pc = perm.reshape(NBLK, BLK)
        for qb in range(NBLK):
            for kt in range(4):
                dm[:, qb * 4 + kt, :] = (
                    pr[qb, kt][:, None] <= pc[qb][None, :])
        m["dmask"] = dm.astype(BF_NP)

        vmp = vm[b][perm]
        vstrip = np.zeros((W,), np.float32)
        for k in range(NBLK):
            w = widths[k]
            if w:
                vstrip[offs[k]:offs[k] + w] = vmp[k * BLK + sA[k]:
                                                  k * BLK + sB[k]]
        m["vmv"] = np.broadcast_to(vstrip, (128, W)).astype(BF_NP).copy()
        m["vml"] = np.broadcast_to(1.0 - vstrip, (128, W)).astype(BF_NP).copy()
        in_maps.append(m)

    meta = dict(sA=sA, sB=sB, widths=widths, offs=offs, W=W, perms=perms)
    return in_maps, meta


# ---------------------------------------------------------------------------
# bass program
# ---------------------------------------------------------------------------

def build_program(meta, debug=False):
    import concourse.tile as tile
    from concourse import bacc, mybir
    from concourse.masks import make_identity

    F32 = mybir.dt.float32
    BF = mybir.dt.bfloat16
    AF = mybir.ActivationFunctionType
    ADD = mybir.AluOpType.add
    sA, sB, widths, offs, W = (meta["sA"], meta["sB"], meta["widths"],
                               meta["offs"], meta["W"])

    nc = bacc.Bacc("TRN2", target_bir_lowering=False, debug=False,
                   num_devices=N_CORES)

    def din(name, shape, dt):
        return nc.dram_tensor(name, shape, dt, kind="ExternalInput").ap()

    h0T = din("h0T", [H, L], F32)
    eT = din("eT", [CH, LE], BF)
    wqkv = {ex: din(f"wqkv_{ex}", [128, KT_H, 1536], BF) for ex in "vl"}
    wdense = {ex: din(f"wdense_{ex}", [128, 4, H], BF) for ex in "vl"}
    wg = {ex: din(f"wg_{ex}", [128, KT_H, ISL], BF) for ex in "vl"}
    wu = {ex: din(f"wu_{ex}", [128, KT_H, ISL], BF) for ex in "vl"}
    wd = {ex: din(f"wd_{ex}", [128, 11, H], BF) for ex in "vl"}
    wcq = din("wcq", [128, KT_H, 256], BF)
    wckv = din("wckv", [128, 8, 512], BF)
    wcd = din("wcd", [128, 2, H], BF)
    ropeq_c = din("ropeq_c", [128, L], BF)
    ropeq_s = din("ropeq_s", [128, L], BF)
    rope_c = din("rope_c", [128, L], BF)
    rope_s = din("rope_s", [128, L], BF)
    dmask = din("dmask", [128, 16, BLK], BF)
    vmv_in = din("vmv", [128, W], BF)
    vml_in = din("vml", [128, W], BF)

    outT = nc.dram_tensor("outT", [H, L], F32, kind="ExternalOutput").ap()
    ar_in = [nc.dram_tensor(f"ar{i}_in", [H, L], F32).ap() for i in range(3)]
    ar_out = [nc.dram_tensor(f"ar{i}_out", [H, L], F32).ap() for i in range(3)]
    hid1 = nc.dram_tensor("hid1", [H, L], F32).ap()
    hid2 = nc.dram_tensor("hid2", [H, L], F32).ap()

    dbg = {}
    if debug:
        for nm, shape in [("dbg_xn1", [H, L]), ("dbg_qhat", [512, L]),
                          ("dbg_khat", [512, L]), ("dbg_ctx", [512, L]),
                          ("dbg_cq", [256, L]), ("dbg_ck", [256, LE]),
                          ("dbg_cctx", [256, L]), ("dbg_hmlp", [ISL, L])]:
            dbg[nm] = nc.dram_tensor(nm, shape, BF, kind="ExternalOutput").ap()

    with tile.TileContext(nc, pool_alloc_mode="queue") as tc, \
            ExitStack() as top:
        def mk_pool(name, bufs, space="SBUF", side=None):
            cm = tc.tile_pool(name=name, bufs=bufs, space=space, side=side)
            return cm, cm.__enter__()

        def close(cm):
            cm.__exit__(None, None, None)

        const_cm, const = mk_pool("const", 1)   # lives until the end
        top.callback(lambda: close(const_cm))
        ones_bf = const.tile([128, 1], BF)
        nc.vector.memset(ones_bf, 1.0)
        ident = const.tile([128, 128], BF)
        make_identity(nc, ident)
        vmv = const.tile([128, W], BF)
        nc.sync.dma_start(out=vmv, in_=vmv_in[:])
        vml = const.tile([128, W], BF)
        nc.sync.dma_start(out=vml, in_=vml_in[:])
        eps_t = const.tile([128, 1], F32)
        nc.vector.memset(eps_t, EPS)

        def expert_mms(ps, wv_t, wl_t, xn_k, xv_k, xl_k, k, klast):
            # start=True clears has_written for the WHOLE bank, so only the
            # first matmul issued to a bank (at k==0) may set it; later
            # regions overwrite-on-clear-bit which is exactly right. stop
            # only on the last matmul to touch the bank.
            plan = {b: [] for b in range(NBLK)}  # (expert, psum_sl, rhs)
            for b in range(NBLK):
                a, e, w, o = sA[b], sB[b], widths[b], int(offs[b])
                if a > 0:
                    plan[b].append(("v", (0, a),
                                    xn_k[:, b * BLK:b * BLK + a]))
                if w > 0:
                    plan[b].append(("v", (a, e), xv_k[:, o:o + w]))
                    plan[b].append(("l", (a, e), xl_k[:, o:o + w]))
                if e < BLK:
                    plan[b].append(("l", (e, BLK),
                                    xn_k[:, b * BLK + e:(b + 1) * BLK]))
            for ex, w_t in (("v", wv_t), ("l", wl_t)):
                for b in range(NBLK):
                    for i, (pex, sl, rhs) in enumerate(plan[b]):
                        if pex != ex:
                            continue
                        st = (k == 0) and i == 0
                        sp = (k == klast) and i == len(plan[b]) - 1
                        nc.tensor.matmul(ps[b][:, sl[0]:sl[1]], w_t, rhs,
                                         start=st, stop=sp)

        def strip_mask(dst_pool, prefix, src_tiles, nk):
            """Make expert-masked strip tiles from src feature-tiles."""
            xv, xl = [], []
            for k in range(nk):
                xvk = dst_pool.tile([128, W], BF, name=f"{prefix}v{k}",
                                    tag=f"{prefix}v{k}")
                xlk = dst_pool.tile([128, W], BF, name=f"{prefix}l{k}",
                                    tag=f"{prefix}l{k}")
                for b in range(NBLK):
                    w, o = widths[b], int(offs[b])
                    if not w:
                        continue
                    src = src_tiles[k][:, b * BLK + sA[b]:b * BLK + sB[b]]
                    nc.vector.tensor_mul(xvk[:, o:o + w], src, vmv[:, o:o + w])
                    nc.vector.tensor_mul(xlk[:, o:o + w], src, vml[:, o:o + w])
                xv.append(xvk)
                xl.append(xlk)
            return xv, xl

        def norm_pass(src_hid, src_ar, dst_hid, xn_pool, xn_name,
                      final_out=None):
            """residual add + RMSNorm -> bf16 xn tiles in xn_pool.
            Reads src_hid (+src_ar) f32 DRAM, optionally writes dst_hid.
            If final_out is set: residual add + store only."""
            with ExitStack() as ctx:
                tp = ctx.enter_context(tc.tile_pool(name=f"nt_{xn_name}",
                                                    bufs=2))
                pp = ctx.enter_context(tc.tile_pool(name=f"np_{xn_name}",
                                                    bufs=1, space="PSUM"))
                var_ps = [pp.tile([1, BLK], F32, name=f"var{n}",
                                  tag=f"var{n}") for n in range(4)]
                reread = src_hid if dst_hid is None else dst_hid
                for k in range(KT_H):
                    ht = tp.tile([128, L], F32, name="ht", tag="ht")
                    nc.sync.dma_start(out=ht,
                                      in_=src_hid[k * 128:(k + 1) * 128, :])
                    if src_ar is not None:
                        at = tp.tile([128, L], F32, name="at", tag="at")
                        nc.sync.dma_start(
                            out=at, in_=src_ar[k * 128:(k + 1) * 128, :])
                        hn = tp.tile([128, L], F32, name="hn", tag="hn")
                        nc.vector.tensor_add(hn, ht, at)
                    else:
                        hn = ht
                    if final_out is not None:
                        nc.sync.dma_start(
                            out=final_out[k * 128:(k + 1) * 128, :], in_=hn)
                        continue
                    if dst_hid is not None:
                        nc.sync.dma_start(
                            out=dst_hid[k * 128:(k + 1) * 128, :], in_=hn)
                    sq = tp.tile([128, L], BF, name="sq", tag="sq")
                    nc.scalar.activation(sq, hn, AF.Square)
                    for n in range(4):
                        nc.tensor.matmul(
                            var_ps[n], ones_bf, sq[:, n * BLK:(n + 1) * BLK],
                            start=(k == 0), stop=(k == KT_H - 1))
                if final_out is not None:
                    return None
                srow = tp.tile([1, L], F32, name="srow", tag="srow", bufs=1)
                for n in range(4):
                    nc.scalar.activation(
                        srow[:, n * BLK:(n + 1) * BLK], var_ps[n], AF.Sqrt,
                        scale=1.0 / H, bias=eps_t[0:1, :])
                sr = tp.tile([1, L], F32, name="sr", tag="sr", bufs=1)
                nc.vector.reciprocal(sr, srow)
                sbc = tp.tile([128, L], F32, name="sbc", tag="sbc", bufs=1)
                nc.gpsimd.partition_broadcast(sbc, sr)
                xn = []
                for k in range(KT_H):
                    ht2 = tp.tile([128, L], F32, name="ht2", tag="ht")
                    nc.sync.dma_start(out=ht2,
                                      in_=reread[k * 128:(k + 1) * 128, :])
                    xk = xn_pool.tile([128, L], BF, name=f"{xn_name}{k}",
                                      tag=f"{xn_name}{k}")
                    nc.vector.tensor_mul(xk, ht2, sbc)
                    xn.append(xk)
                return xn

        # ================= phase 1: input norm =================
        xn1_cm, xn1_pool = mk_pool("xn1_pool", 1)
        xn1 = norm_pass(h0T, None, None, xn1_pool, "xn1")
        xv1, xl1 = strip_mask(xn1_pool, "x1", xn1, KT_H)
        if debug:
            for k in range(KT_H):
                nc.sync.dma_start(
                    out=dbg["dbg_xn1"][k * 128:(k + 1) * 128, :], in_=xn1[k])

        rope_cm, rope = mk_pool("rope", 1)
        rq_c = rope.tile([128, L], BF, name="rq_c")
        nc.sync.dma_start(out=rq_c, in_=ropeq_c[:])
        rq_s = rope.tile([128, L], BF, name="rq_s")
        nc.sync.dma_start(out=rq_s, in_=ropeq_s[:])
        rk_c = rope.tile([128, L], BF, name="rk_c")
        nc.sync.dma_start(out=rk_c, in_=rope_c[:])
        rk_s = rope.tile([128, L], BF, name="rk_s")
        nc.sync.dma_start(out=rk_s, in_=rope_s[:])

        qk_cm, qk_pool = mk_pool("qk_pool", 1, side="right")
        qhat = [qk_pool.tile([128, L], BF, name=f"qhat{h}", tag=f"qhat{h}")
                for h in range(4)]
        khat = [qk_pool.tile([128, L], BF, name=f"khat{h}", tag=f"khat{h}")
                for h in range(4)]
        vt = [qk_pool.tile([128, KT_H, 128], BF, name=f"vt{h}", tag=f"vt{h}")
              for h in range(4)]

        # ================= phase 2: QKV =================
        with ExitStack() as ctx:
            wp = ctx.enter_context(tc.tile_pool(name="qkv_w", bufs=2))
            pp = ctx.enter_context(tc.tile_pool(name="qkv_ps", bufs=6,
                                                space="PSUM"))
            ep = ctx.enter_context(tc.tile_pool(name="qkv_e", bufs=2))
            for m in range(12):
                wv_s = wp.tile([128, KT_H, 128], BF, name="wv_s", tag="wv_s")
                nc.sync.dma_start(out=wv_s,
                                  in_=wqkv["v"][:, :, m * 128:(m + 1) * 128])
                wl_s = wp.tile([128, KT_H, 128], BF, name="wl_s", tag="wl_s")
                nc.sync.dma_start(out=wl_s,
                                  in_=wqkv["l"][:, :, m * 128:(m + 1) * 128])
                ps = [pp.tile([128, BLK], F32, name="qkvps", tag="qkvps")
                      for _ in range(NBLK)]
                for k in range(KT_H):
                    expert_mms(ps, wv_s[:, k, :], wl_s[:, k, :],
                               xn1[k], xv1[k], xl1[k], k, KT_H - 1)
                if m < 8:   # q or k head -> rope
                    h = m % 4
                    dst = qhat[h] if m < 4 else khat[h]
                    tab_c = rq_c if m < 4 else rk_c
                    tab_s = rq_s if m < 4 else rk_s
                    raw = ep.tile([128, L], BF, name="rawqk", tag="rawqk")
                    for b in range(NBLK):
                        nc.scalar.activation(
                            raw[:, b * BLK:(b + 1) * BLK], ps[b], AF.Copy)
                    shf = ep.tile([128, L], BF, name="shf", tag="shf")
                    nc.sync.dma_start(out=shf[0:64, :], in_=raw[64:128, :])
                    nc.sync.dma_start(out=shf[64:128, :], in_=raw[0:64, :])
                    t1 = ep.tile([128, L], BF, name="ropet1", tag="ropet1")
                    nc.vector.tensor_mul(t1, raw, tab_c)
                    t2 = ep.tile([128, L], BF, name="ropet2", tag="ropet2")
                    nc.vector.tensor_mul(t2, shf, tab_s)
                    nc.vector.tensor_add(dst, t1, t2)
                else:       # v head -> copy + per-ktile transpose
                    h = m - 8
                    vh = ep.tile([128, L], BF, name="vh", tag="vh")
                    for b in range(NBLK):
                        nc.scalar.activation(
                            vh[:, b * BLK:(b + 1) * BLK], ps[b], AF.Copy)
                    with tc.tile_pool(name="qkv_tp", bufs=2,
                                      space="PSUM") as tpp:
                        for kt in range(KT_H):
                            tps = tpp.tile([128, 128], BF, name="tps",
                                           tag="tps")
                            nc.tensor.transpose(
                                tps, vh[:, kt * 128:(kt + 1) * 128], ident)
                            nc.scalar.activation(vt[h][:, kt, :], tps,
                                                 AF.Copy)
        close(rope_cm)
        close(xn1_cm)
        if debug:
            for h in range(4):
                nc.sync.dma_start(
                    out=dbg["dbg_qhat"][h * 128:(h + 1) * 128, :], in_=qhat[h])
                nc.sync.dma_start(
                    out=dbg["dbg_khat"][h * 128:(h + 1) * 128, :], in_=khat[h])

        # ================= phase 3: self attention =================
        ctx_cm, ctx_pool = mk_pool("ctx_pool", 1)
        dm_cm, dmp = mk_pool("dmsk", 1)
        dm = dmp.tile([128, 16, BLK], BF, name="dm")
        nc.sync.dma_start(out=dm, in_=dmask[:])
        ctxh = [ctx_pool.tile([128, L], BF, name=f"ctx{h}", tag=f"ctx{h}")
                for h in range(4)]
        with ExitStack() as ctx:
            sp = ctx.enter_context(tc.tile_pool(name="att_s", bufs=3,
                                                space="PSUM"))
            up = ctx.enter_context(tc.tile_pool(name="att_u", bufs=2,
                                                space="PSUM"))
            dp = ctx.enter_context(tc.tile_pool(name="att_d", bufs=2,
                                                space="PSUM"))
            ep = ctx.enter_context(tc.tile_pool(name="att_e", bufs=6))
            for h in range(4):
                for qb in range(NBLK):
                    u_ps = up.tile([128, BLK], F32, name="u_ps", tag="u_ps")
                    d_ps = dp.tile([1, BLK], F32, name="d_ps", tag="d_ps")
                    nkt = qb * 4 + 4
                    for kt in range(nkt):
                        s_ps = sp.tile([128, BLK], F32, name="s_ps",
                                       tag="s_ps")
                        nc.tensor.matmul(
                            s_ps, khat[h][:, kt * 128:(kt + 1) * 128],
                            qhat[h][:, qb * BLK:(qb + 1) * BLK])
                        e_t = ep.tile([128, BLK], BF, name="e_t", tag="e_t")
                        nc.scalar.activation(e_t, s_ps, AF.Exp)
                        if kt // 4 == qb:
                            nc.vector.tensor_mul(
                                e_t, e_t, dm[:, qb * 4 + (kt % 4), :])
                        nc.tensor.matmul(u_ps, vt[h][:, kt, :], e_t,
                                         start=(kt == 0), stop=(kt == nkt - 1))
                        nc.tensor.matmul(d_ps, ones_bf, e_t,
                                         start=(kt == 0), stop=(kt == nkt - 1))
                    rc = ep.tile([1, BLK], F32, name="rc", tag="rc")
                    nc.vector.reciprocal(rc, d_ps)
                    rb = ep.tile([128, BLK], F32, name="rb", tag="rb")
                    nc.gpsimd.partition_broadcast(rb, rc)
                    nc.vector.tensor_mul(
                        ctxh[h][:, qb * BLK:(qb + 1) * BLK], u_ps, rb)
        close(dm_cm)
        close(qk_cm)
        if debug:
            for h in range(4):
                nc.sync.dma_start(
                    out=dbg["dbg_ctx"][h * 128:(h + 1) * 128, :], in_=ctxh[h])

        ctxv, ctxl = strip_mask(ctx_pool, "cx", ctxh, NBLK)

        # ================= phase 4: dense + AR0 =================
        with ExitStack() as ctx:
            wp = ctx.enter_context(tc.tile_pool(name="dns_w", bufs=1))
            pp = ctx.enter_context(tc.tile_pool(name="dns_ps", bufs=8,
                                                space="PSUM"))
            ep = ctx.enter_context(tc.tile_pool(name="dns_e", bufs=3))
            wv_s = wp.tile([128, 4, H], BF, name="wdv")
            nc.sync.dma_start(out=wv_s, in_=wdense["v"][:])
            wl_s = wp.tile([128, 4, H], BF, name="wdl")
            nc.sync.dma_start(out=wl_s, in_=wdense["l"][:])
            for m in range(KT_H):
                ps = [pp.tile([128, BLK], F32, name="dnsps", tag="dnsps")
                      for _ in range(NBLK)]
                for k in range(NBLK):
                    expert_mms(ps, wv_s[:, k, m * 128:(m + 1) * 128],
                               wl_s[:, k, m * 128:(m + 1) * 128],
                               ctxh[k], ctxv[k], ctxl[k], k, NBLK - 1)
                st = ep.tile([128, L], F32, name="dnsst", tag="dnsst")
                for b in range(NBLK):
                    nc.scalar.activation(st[:, b * BLK:(b + 1) * BLK],
                                         ps[b], AF.Copy)
                nc.sync.dma_start(out=ar_in[0][m * 128:(m + 1) * 128, :],
                                  in_=st)
        close(ctx_cm)
        nc.gpsimd.collective_compute(
            "AllReduce", ADD, replica_groups=GROUPS,
            ins=[ar_in[0][:]], outs=[ar_out[0][:]])

        # ================= phase 5: residual1 + norm2 (ci) =================
        ci_cm, ci_pool = mk_pool("ci_pool", 1, side="right")
        ci = norm_pass(h0T, ar_out[0], hid1, ci_pool, "ci")

        # ================= phase 6: cross attention =================
        crs_cm, crs = mk_pool("crs_a", 1)
        with ExitStack() as ctx:
            wp = ctx.enter_context(tc.tile_pool(name="crs_w", bufs=1))
            ep = ctx.enter_context(tc.tile_pool(name="crs_e", bufs=3))

            et = []
            for k in range(8):
                etk = crs.tile([128, LE], BF, name=f"et{k}", tag=f"et{k}")
                nc.sync.dma_start(out=etk, in_=eT[k * 128:(k + 1) * 128, :])
                et.append(etk)
            wkv_s = wp.tile([128, 8, 512], BF, name="wkv_s")
            nc.sync.dma_start(out=wkv_s, in_=wckv[:])
            ckh = [crs.tile([128, LE], BF, name=f"ckh{t}", tag=f"ckh{t}")
                   for t in range(2)]
            cvh = [crs.tile([128, LE], BF, name=f"cvh{t}", tag=f"cvh{t}")
                   for t in range(2)]
            pp_ckv_cm = tc.tile_pool(name="ps_ckv", bufs=4, space="PSUM")
            pp = pp_ckv_cm.__enter__()
            for m in range(4):  # rows: ck tile0, ck tile1, cv tile0, cv tile1
                dst = ckh[m] if m < 2 else cvh[m - 2]
                ps = [pp.tile([128, BLK], F32, name="ckvps", tag="ckvps")
                      for _ in range(2)]
                for k in range(8):
                    for bb in range(2):
                        nc.tensor.matmul(
                            ps[bb], wkv_s[:, k, m * 128:(m + 1) * 128],
                            et[k][:, bb * BLK:(bb + 1) * BLK],
                            start=(k == 0), stop=(k == 7))
                for bb in range(2):
                    nc.scalar.activation(dst[:, bb * BLK:(bb + 1) * BLK],
                                         ps[bb], AF.Copy)
            pp_ckv_cm.__exit__(None, None, None)
            vc = [crs.tile([128, 8, CHD], BF, name=f"vc{h}", tag=f"vc{h}")
                  for h in range(4)]
            pp_tp_cm = tc.tile_pool(name="ps_ctp", bufs=2, space="PSUM")
            tpp = pp_tp_cm.__enter__()
            for h in range(4):
                t, base = h // 2, (h % 2) * 64
                for kt in range(8):
                    tps = tpp.tile([128, CHD], BF, name="ctps", tag="ctps")
                    nc.tensor.transpose(
                        tps, cvh[t][base:base + 64, kt * 128:(kt + 1) * 128],
                        ident[base:base + 64, base:base + 64])
                    nc.scalar.activation(vc[h][:, kt, :], tps, AF.Copy)
            if debug:
                for t in range(2):
                    nc.sync.dma_start(
                        out=dbg["dbg_ck"][t * 128:(t + 1) * 128, :],
                        in_=ckh[t])

            wq_s = wp.tile([128, KT_H, 256], BF, name="wq_s")
            nc.sync.dma_start(out=wq_s, in_=wcq[:])
            pp_tp_cm.__exit__(None, None, None)
            cqh = [crs.tile([128, L], BF, name=f"cqh{t}", tag=f"cqh{t}")
                   for t in range(2)]
            pp_cq_cm = tc.tile_pool(name="ps_cq", bufs=4, space="PSUM")
            pp = pp_cq_cm.__enter__()
            for m in range(2):
                ps = [pp.tile([128, BLK], F32, name="cqps", tag="cqps")
                      for _ in range(NBLK)]
                for k in range(KT_H):
                    for bb in range(NBLK):
                        nc.tensor.matmul(
                            ps[bb], wq_s[:, k, m * 128:(m + 1) * 128],
                            ci[k][:, bb * BLK:(bb + 1) * BLK],
                            start=(k == 0), stop=(k == KT_H - 1))
                for bb in range(NBLK):
                    nc.scalar.activation(cqh[m][:, bb * BLK:(bb + 1) * BLK],
                                         ps[bb], AF.Copy)
            close(ci_cm)
            if debug:
                for t in range(2):
                    nc.sync.dma_start(
                        out=dbg["dbg_cq"][t * 128:(t + 1) * 128, :],
                        in_=cqh[t])

            pp_cq_cm.__exit__(None, None, None)
            cctx = [crs.tile([64, L], BF, name=f"cctx{h}", tag=f"cctx{h}")
                    for h in range(4)]
            pp_fl_cm = tc.tile_pool(name="ps_cfl", bufs=2, space="PSUM")
            pp = pp_fl_cm.__enter__()
            for h in range(4):
                t, base = h // 2, (h % 2) * 64
                for qb in range(NBLK):
                    u_ps = pp.tile([64, BLK], F32, name="cu_ps", tag="cu_ps")
                    d_ps = pp.tile([1, BLK], F32, name="cd_ps", tag="cd_ps")
                    for kt in range(8):
                        s_ps = pp.tile([128, BLK], F32, name="cs_ps",
                                       tag="cs_ps", bufs=3)
                        nc.tensor.matmul(
                            s_ps,
                            ckh[t][base:base + 64, kt * 128:(kt + 1) * 128],
                            cqh[t][base:base + 64, qb * BLK:(qb + 1) * BLK])
                        e_t = ep.tile([128, BLK], BF, name="ce_t", tag="ce_t")
                        nc.scalar.activation(e_t, s_ps, AF.Exp)
                        nc.tensor.matmul(u_ps, vc[h][:, kt, :], e_t,
                                         start=(kt == 0), stop=(kt == 7))
                        nc.tensor.matmul(d_ps, ones_bf, e_t,
                                         start=(kt == 0), stop=(kt == 7))
                    rc = ep.tile([1, BLK], F32, name="crc", tag="crc")
                    nc.vector.reciprocal(rc, d_ps)
                    rb = ep.tile([64, BLK], F32, name="crb", tag="crb")
                    nc.gpsimd.partition_broadcast(rb, rc)
                    nc.vector.tensor_mul(
                        cctx[h][:, qb * BLK:(qb + 1) * BLK], u_ps, rb)
            cpk = [crs.tile([128, L], BF, name=f"cpk{t}", tag=f"cpk{t}")
                   for t in range(2)]
            for t in range(2):
                nc.sync.dma_start(out=cpk[t][0:64, :], in_=cctx[2 * t])
                nc.sync.dma_start(out=cpk[t][64:128, :], in_=cctx[2 * t + 1])
            if debug:
                for t in range(2):
                    nc.sync.dma_start(
                        out=dbg["dbg_cctx"][t * 128:(t + 1) * 128, :],
                        in_=cpk[t])

            pp_fl_cm.__exit__(None, None, None)
            wcd_s = wp.tile([128, 2, H], BF, name="wcd_s")
            nc.sync.dma_start(out=wcd_s, in_=wcd[:])
            pp_cd_cm = tc.tile_pool(name="ps_cd", bufs=8, space="PSUM")
            pp = pp_cd_cm.__enter__()
            for m in range(KT_H):
                ps = [pp.tile([128, BLK], F32, name="cdps", tag="cdps")
                      for _ in range(NBLK)]
                for k in range(2):
                    for bb in range(NBLK):
                        nc.tensor.matmul(
                            ps[bb], wcd_s[:, k, m * 128:(m + 1) * 128],
                            cpk[k][:, bb * BLK:(bb + 1) * BLK],
                            start=(k == 0), stop=(k == 1))
                st = ep.tile([128, L], F32, name="cdst", tag="cdst")
                for bb in range(NBLK):
                    nc.scalar.activation(st[:, bb * BLK:(bb + 1) * BLK],
                                         ps[bb], AF.Copy)
                nc.sync.dma_start(out=ar_in[1][m * 128:(m + 1) * 128, :],
                                  in_=st)
            pp_cd_cm.__exit__(None, None, None)
        close(crs_cm)
        nc.gpsimd.collective_compute(
            "AllReduce", ADD, replica_groups=GROUPS,
            ins=[ar_in[1][:]], outs=[ar_out[1][:]])

        # ================= phase 7: residual2 + norm3 (mi) =================
        mi_cm, mi_pool = mk_pool("mi_pool", 1, side="right")
        mi = norm_pass(hid1, ar_out[1], hid2, mi_pool, "mi")
        miv, mil = strip_mask(mi_pool, "m3", mi, KT_H)

        # ================= phase 8: MLP =================
        hm_cm, hm_pool = mk_pool("hm_pool", 1)
        hm = [hm_pool.tile([128, L], BF, name=f"hm{m}", tag=f"hm{m}")
              for m in range(11)]
        with ExitStack() as ctx:
            wp_cm = tc.tile_pool(name="mlp_w", bufs=2)
            wp = wp_cm.__enter__()
            ep_cm = tc.tile_pool(name="mlp_e", bufs=3)
            ep = ep_cm.__enter__()
            gp_cm = tc.tile_pool(name="mlp_gps", bufs=4, space="PSUM")
            gp = gp_cm.__enter__()
            upp_cm = tc.tile_pool(name="mlp_ups", bufs=4, space="PSUM")
            upp = upp_cm.__enter__()
            for m in range(11):
                wgv = wp.tile([128, KT_H, 128], BF, name="wgv", tag="wgv")
                nc.sync.dma_start(out=wgv,
                                  in_=wg["v"][:, :, m * 128:(m + 1) * 128])
                wgl = wp.tile([128, KT_H, 128], BF, name="wgl", tag="wgl")
                nc.sync.dma_start(out=wgl,
                                  in_=wg["l"][:, :, m * 128:(m + 1) * 128])
                wuv = wp.tile([128, KT_H, 128], BF, name="wuv", tag="wuv")
                nc.sync.dma_start(out=wuv,
                                  in_=wu["v"][:, :, m * 128:(m + 1) * 128])
                wul = wp.tile([128, KT_H, 128], BF, name="wul", tag="wul")
                nc.sync.dma_start(out=wul,
                                  in_=wu["l"][:, :, m * 128:(m + 1) * 128])
                gps = [gp.tile([128, BLK], F32, name="gps", tag="gps")
                       for _ in range(NBLK)]
                ups = [upp.tile([128, BLK], F32, name="ups", tag="ups")
                       for _ in range(NBLK)]
                for k in range(KT_H):
                    expert_mms(gps, wgv[:, k, :], wgl[:, k, :],
                               mi[k], miv[k], mil[k], k, KT_H - 1)
                    expert_mms(ups, wuv[:, k, :], wul[:, k, :],
                               mi[k], miv[k], mil[k], k, KT_H - 1)
                for b in range(NBLK):
                    sg = ep.tile([128, BLK], BF, name="sg", tag="sg")
                    nc.scalar.activation(sg, gps[b], AF.Silu)
                    nc.vector.tensor_mul(
                        hm[m][:, b * BLK:(b + 1) * BLK], sg, ups[b])
            upp_cm.__exit__(None, None, None)
            gp_cm.__exit__(None, None, None)
            ep_cm.__exit__(None, None, None)
            wp_cm.__exit__(None, None, None)
            close(mi_cm)
            hmv, hml = strip_mask(hm_pool, "hm", hm, 11)
            wp = ctx.enter_context(tc.tile_pool(name="mlp_w2", bufs=2))
            ep = ctx.enter_context(tc.tile_pool(name="mlp_e2", bufs=2))
            if debug:
                for m in range(11):
                    nc.sync.dma_start(
                        out=dbg["dbg_hmlp"][m * 128:(m + 1) * 128, :],
                        in_=hm[m])
            dw_cm = tc.tile_pool(name="mlp_dps", bufs=8, space="PSUM")
            dwp = dw_cm.__enter__()
            for mc in range(8):
                wdv = wp.tile([128, 11, 256], BF, name="wdnv", tag="wdnv")
                nc.sync.dma_start(out=wdv,
                                  in_=wd["v"][:, :, mc * 256:(mc + 1) * 256])
                wdl = wp.tile([128, 11, 256], BF, name="wdnl", tag="wdnl")
                nc.sync.dma_start(out=wdl,
                                  in_=wd["l"][:, :, mc * 256:(mc + 1) * 256])
                for mloc in range(2):
                    m = mc * 2 + mloc
                    ps = [dwp.tile([128, BLK], F32, name="dwps", tag="dwps")
                          for _ in range(NBLK)]
                    for k in range(11):
                        expert_mms(ps, wdv[:, k, mloc * 128:(mloc + 1) * 128],
                                   wdl[:, k, mloc * 128:(mloc + 1) * 128],
                                   hm[k], hmv[k], hml[k], k, 10)
                    st = ep.tile([128, L], F32, name="dwst", tag="dwst")
                    for b in range(NBLK):
                        nc.scalar.activation(st[:, b * BLK:(b + 1) * BLK],
                                             ps[b], AF.Copy)
                    nc.sync.dma_start(out=ar_in[2][m * 128:(m + 1) * 128, :],
                                      in_=st)
            dw_cm.__exit__(None, None, None)
        close(hm_cm)
        nc.gpsimd.collective_compute(
            "AllReduce", ADD, replica_groups=GROUPS,
            ins=[ar_in[2][:]], outs=[ar_out[2][:]])

        # ================= phase 9: residual3 -> out =================
        norm_pass(hid2, ar_out[2], None, None, "fin", final_out=outT)

    nc.compile()
    return nc


def _get_program(meta, debug):
    key = (tuple(meta["sA"]), tuple(meta["sB"]), debug)
    if key not in _CACHE:
        _CACHE[key] = build_program(meta, debug)
    return _CACHE[key]


def run(inputs, debug=False, trace=False):
    from concourse.bass_utils import run_bass_kernel_spmd
    in_maps, meta = host_prep(inputs)
    nc = _get_program(meta, debug)
    res = run_bass_kernel_spmd(nc, in_maps, list(range(N_CORES)),
                               trace=trace)
    outs = []
    for b in range(B):
        perm = meta["perms"][b]
        oT = res.results[b * TP]["outT"]          # [H, L] f32, permuted
        ob = np.empty((L, H), np.float32)
        ob[perm] = np.ascontiguousarray(oT.T)
        outs.append(ob)
    out = np.stack(outs).astype(np.float32)
    return out, res, meta


def kernel(**inputs) -> np.ndarray:
    out, _, _ = run(inputs)
    return out


# revision 21
# speedup vs baseline: 1.4233x; 1.4233x over previous
"""Trainium2 Bass kernel for nn_CogAgentDecoderLayer (moe_routing).

DP2 x TP4 across 8 NeuronCores: cores 0-3 batch 0, cores 4-7 batch 1;
within a group, tensor-parallel over heads / intermediate dim with 3
grouped AllReduces. Device activations are feature-major
([features, tokens]); host transposes inputs/outputs and pre-casts
weights to bf16. Tokens are permuted within each 512-block so
vision-expert tokens come first, letting both experts' matmuls run
packed, with a small data-masked overlap strip keeping the program
SPMD-identical across batches.
"""

import numpy as np
import ml_dtypes
from contextlib import ExitStack

H = 2048
NH = 16
HD = 128
CH = 1024
CC = 1024
CHD = 64
ISL = 1408          # per-core intermediate slice (padded to 11*128)
B = 2
L = 2048
LE = 1024
EPS = 1e-6
N_CORES = 8
TP = 4
GROUPS = [[0, 1, 2, 3], [4, 5, 6, 7]]
NBLK = 4
BLK = 512
KT_H = H // 128     # 16
PACKED = True
BF_NP = ml_dtypes.bfloat16

_CACHE = {}


def _tile128(w):
    """[K, M] -> [128, K//128, M] contiguous bf16 (ki = partition)."""
    K, M = w.shape
    return np.ascontiguousarray(
        w.reshape(K // 128, 128, M).transpose(1, 0, 2)
    ).astype(BF_NP)


def host_prep(inputs):
    hs = np.asarray(inputs["hidden_states"], np.float32)
    enc = np.asarray(inputs["encoder_outputs"], np.float32)
    tt = np.asarray(inputs["token_type_ids"])
    pos = np.asarray(inputs["position_ids"])[0]

    vm = np.zeros((B, L), bool)
    vm[:, :-1] = (tt[:, :-1] == 1) & (tt[:, 1:] == 1)

    perms = []
    for b in range(B):
        parts = []
        for blk in range(NBLK):
            idx = np.arange(blk * BLK, (blk + 1) * BLK)
            parts.append(np.concatenate([idx[vm[b, idx]], idx[~vm[b, idx]]]))
        perms.append(np.concatenate(parts))
    nv = [[int(vm[b, blk * BLK:(blk + 1) * BLK].sum()) for blk in range(NBLK)]
          for b in range(B)]
    if PACKED:
        sA = [min(nv[0][k], nv[1][k]) for k in range(NBLK)]
        sB = [max(nv[0][k], nv[1][k]) for k in range(NBLK)]
    else:
        perms = [np.arange(L), np.arange(L)]
        sA = [0] * NBLK
        sB = [BLK] * NBLK
    widths = [sB[k] - sA[k] for k in range(NBLK)]
    offs = np.concatenate([[0], np.cumsum(widths)]).astype(int)
    W = max(int(offs[-1]), 1)

    inv = 1.0 / (10000.0 ** (np.arange(0, HD, 2, dtype=np.float32) / HD))
    freqs = np.outer(pos.astype(np.float32), inv)        # [L, 64]
    Cfull = np.concatenate([np.cos(freqs), np.cos(freqs)], 1).T   # [128, L]
    Sfull = np.concatenate([-np.sin(freqs), np.sin(freqs)], 1).T  # [128, L]
    qscale = 1.0 / np.sqrt(HD)

    lnin = np.asarray(inputs["ln_in_w"], np.float32)
    lnc = np.asarray(inputs["ln_post_cross_w"], np.float32)
    lna = np.asarray(inputs["ln_post_attn_w"], np.float32)

    in_maps = []
    for core in range(N_CORES):
        b, r = core // TP, core % TP
        perm = perms[b]
        m = {}
        m["h0T"] = np.ascontiguousarray(hs[b][perm].T)           # [H, L] f32
        m["eT"] = np.ascontiguousarray(enc[b].T).astype(BF_NP)   # [CH, LE]

        qkv_cols = np.concatenate([
            np.arange(r * 512, (r + 1) * 512),
            H + np.arange(r * 512, (r + 1) * 512),
            2 * H + np.arange(r * 512, (r + 1) * 512)])
        for ex in ("v", "l"):
            wq = np.asarray(inputs[f"{ex}_qkv"], np.float32)
            m[f"wqkv_{ex}"] = _tile128((wq * lnin[:, None])[:, qkv_cols])
            wdn = np.asarray(inputs[f"{ex}_dense"], np.float32)
            m[f"wdense_{ex}"] = _tile128(wdn[r * 512:(r + 1) * 512, :])
            wgt = np.asarray(inputs[f"{ex}_gate"], np.float32)
            wup = np.asarray(inputs[f"{ex}_up"], np.float32)
            wdw = np.asarray(inputs[f"{ex}_down"], np.float32)
            isl = np.arange(r * 1376, (r + 1) * 1376)
            gpad = np.zeros((H, ISL), np.float32)
            gpad[:, :1376] = (wgt * lna[:, None])[:, isl]
            m[f"wg_{ex}"] = _tile128(gpad)
            upad = np.zeros((H, ISL), np.float32)
            upad[:, :1376] = (wup * lna[:, None])[:, isl]
            m[f"wu_{ex}"] = _tile128(upad)
            dpad = np.zeros((ISL, H), np.float32)
            dpad[:1376, :] = wdw[isl, :]
            m[f"wd_{ex}"] = _tile128(dpad)

        wcq = np.asarray(inputs["cq_w"], np.float32)
        m["wcq"] = _tile128((wcq * lnc[:, None] / np.sqrt(CHD))
                            [:, r * 256:(r + 1) * 256])
        wckv = np.asarray(inputs["ckv_w"], np.float32)
        ckv_cols = np.concatenate([np.arange(r * 256, (r + 1) * 256),
                                   CC + np.arange(r * 256, (r + 1) * 256)])
        m["wckv"] = _tile128(wckv[:, ckv_cols])
        wcd = np.asarray(inputs["cdense_w"], np.float32)
        m["wcd"] = _tile128(wcd[r * 256:(r + 1) * 256, :])

        m["ropeq_c"] = (Cfull[:, perm] * qscale).astype(BF_NP).copy()
        m["ropeq_s"] = (Sfull[:, perm] * qscale).astype(BF_NP).copy()
        m["rope_c"] = Cfull[:, perm].astype(BF_NP).copy()
        m["rope_s"] = Sfull[:, perm].astype(BF_NP).copy()

        dm = np.zeros((128, 16, BLK), np.float32)
        pr = perm.reshape(NBLK, 4, 128)
        pc = perm.reshape(NBLK, BLK)
        for qb in range(NBLK):
            for kt in range(4):
                dm[:, qb * 4 + kt, :] = (
                    pr[qb, kt][:, None] <= pc[qb][None, :])
        m["dmask"] = dm.astype(BF_NP)

        vmp = vm[b][perm]
        vstrip = np.zeros((W,), np.float32)
        for k in range(NBLK):
            w = widths[k]
            if w:
                vstrip[offs[k]:offs[k] + w] = vmp[k * BLK + sA[k]:
                                                  k * BLK + sB[k]]
        m["vmv"] = np.broadcast_to(vstrip, (128, W)).astype(BF_NP).copy()
        m["vml"] = np.broadcast_to(1.0 - vstrip, (128, W)).astype(BF_NP).copy()
        in_maps.append(m)

    meta = dict(sA=sA, sB=sB, widths=widths, offs=offs, W=W, perms=perms)
    return in_maps, meta


# ---------------------------------------------------------------------------
# bass program
# ---------------------------------------------------------------------------

def build_program(meta, debug=False):
    import concourse.tile as tile
    from concourse import bacc, mybir
    from concourse.masks import make_identity

    F32 = mybir.dt.float32
    BF = mybir.dt.bfloat16
    AF = mybir.ActivationFunctionType
    ADD = mybir.AluOpType.add
    sA, sB, widths, offs, W = (meta["sA"], meta["sB"], meta["widths"],
                               meta["offs"], meta["W"])

    nc = bacc.Bacc("TRN2", target_bir_lowering=False, debug=False,
                   num_devices=N_CORES)

    def din(name, shape, dt):
        return nc.dram_tensor(name, shape, dt, kind="ExternalInput").ap()

    h0T = din("h0T", [H, L], F32)
    eT = din("eT", [CH, LE], BF)
    wqkv = {ex: din(f"wqkv_{ex}", [128, KT_H, 1536], BF) for ex in "vl"}
    wdense = {ex: din(f"wdense_{ex}", [128, 4, H], BF) for ex in "vl"}
    wg = {ex: din(f"wg_{ex}", [128, KT_H, ISL], BF) for ex in "vl"}
    wu = {ex: din(f"wu_{ex}", [128, KT_H, ISL], BF) for ex in "vl"}
    wd = {ex: din(f"wd_{ex}", [128, 11, H], BF) for ex in "vl"}
    wcq = din("wcq", [128, KT_H, 256], BF)
    wckv = din("wckv", [128, 8, 512], BF)
    wcd = din("wcd", [128, 2, H], BF)
    ropeq_c = din("ropeq_c", [128, L], BF)
    ropeq_s = din("ropeq_s", [128, L], BF)
    rope_c = din("rope_c", [128, L], BF)
    rope_s = din("rope_s", [128, L], BF)
    dmask = din("dmask", [128, 16, BLK], BF)
    vmv_in = din("vmv", [128, W], BF)
    vml_in = din("vml", [128, W], BF)

    outT = nc.dram_tensor("outT", [H, L], F32, kind="ExternalOutput").ap()
    ar_in = [nc.dram_tensor(f"ar{i}_in", [H, L], BF).ap() for i in range(3)]
    ar_out = [nc.dram_tensor(f"ar{i}_out", [H, L], BF).ap() for i in range(3)]
    hid1 = nc.dram_tensor("hid1", [H, L], F32).ap()
    hid2 = nc.dram_tensor("hid2", [H, L], F32).ap()

    dbg = {}
    if debug:
        for nm, shape in [("dbg_xn1", [H, L]), ("dbg_qhat", [512, L]),
                          ("dbg_khat", [512, L]), ("dbg_ctx", [512, L]),
                          ("dbg_cq", [256, L]), ("dbg_ck", [256, LE]),
                          ("dbg_cctx", [256, L]), ("dbg_hmlp", [ISL, L])]:
            dbg[nm] = nc.dram_tensor(nm, shape, BF, kind="ExternalOutput").ap()

    with tile.TileContext(nc, pool_alloc_mode="queue") as tc, \
            ExitStack() as top:
        def mk_pool(name, bufs, space="SBUF", side=None):
            cm = tc.tile_pool(name=name, bufs=bufs, space=space, side=side)
            return cm, cm.__enter__()

        def close(cm):
            cm.__exit__(None, None, None)

        const_cm, const = mk_pool("const", 1)   # lives until the end
        top.callback(lambda: close(const_cm))
        ones_bf = const.tile([128, 1], BF)
        nc.vector.memset(ones_bf, 1.0)
        ident = const.tile([128, 128], BF)
        make_identity(nc, ident)
        vmv = const.tile([128, W], BF)
        nc.sync.dma_start(out=vmv, in_=vmv_in[:])
        vml = const.tile([128, W], BF)
        nc.sync.dma_start(out=vml, in_=vml_in[:])
        eps_t = const.tile([128, 1], F32)
        nc.vector.memset(eps_t, EPS)

        def expert_mms(ps, wv_t, wl_t, xn_k, xv_k, xl_k, k, klast):
            # start=True clears has_written for the WHOLE bank, so only the
            # first matmul issued to a bank (at k==0) may set it; later
            # regions overwrite-on-clear-bit which is exactly right. stop
            # only on the last matmul to touch the bank.
            plan = {b: [] for b in range(NBLK)}  # (expert, psum_sl, rhs)
            for b in range(NBLK):
                a, e, w, o = sA[b], sB[b], widths[b], int(offs[b])
                if a > 0:
                    plan[b].append(("v", (0, a),
                                    xn_k[:, b * BLK:b * BLK + a]))
                if w > 0:
                    plan[b].append(("v", (a, e), xv_k[:, o:o + w]))
                    plan[b].append(("l", (a, e), xl_k[:, o:o + w]))
                if e < BLK:
                    plan[b].append(("l", (e, BLK),
                                    xn_k[:, b * BLK + e:(b + 1) * BLK]))
            for ex, w_t in (("v", wv_t), ("l", wl_t)):
                for b in range(NBLK):
                    for i, (pex, sl, rhs) in enumerate(plan[b]):
                        if pex != ex:
                            continue
                        st = (k == 0) and i == 0
                        sp = (k == klast) and i == len(plan[b]) - 1
                        nc.tensor.matmul(ps[b][:, sl[0]:sl[1]], w_t, rhs,
                                         start=st, stop=sp)

        def ar_chunk(i, j):
            r0, r1 = j * 512, (j + 1) * 512
            nc.gpsimd.collective_compute(
                "AllReduce", ADD, replica_groups=GROUPS,
                ins=[ar_in[i][r0:r1, :]], outs=[ar_out[i][r0:r1, :]])

        def strip_mask(dst_pool, prefix, src_tiles, nk):
            """Make expert-masked strip tiles from src feature-tiles."""
            xv, xl = [], []
            for k in range(nk):
                xvk = dst_pool.tile([128, W], BF, name=f"{prefix}v{k}",
                                    tag=f"{prefix}v{k}")
                xlk = dst_pool.tile([128, W], BF, name=f"{prefix}l{k}",
                                    tag=f"{prefix}l{k}")
                for b in range(NBLK):
                    w, o = widths[b], int(offs[b])
                    if not w:
                        continue
                    src = src_tiles[k][:, b * BLK + sA[b]:b * BLK + sB[b]]
                    nc.vector.tensor_mul(xvk[:, o:o + w], src, vmv[:, o:o + w])
                    nc.vector.tensor_mul(xlk[:, o:o + w], src, vml[:, o:o + w])
                xv.append(xvk)
                xl.append(xlk)
            return xv, xl

        def norm_pass(src_hid, src_ar, dst_hid, xn_pool, xn_name,
                      final_out=None, tp_ext=None):
            """residual add + RMSNorm -> bf16 xn tiles in xn_pool.
            Reads src_hid (+src_ar) f32 DRAM, optionally writes dst_hid.
            If final_out is set: residual add + store only."""
            with ExitStack() as ctx:
                tp = tp_ext if tp_ext is not None else ctx.enter_context(
                    tc.tile_pool(name=f"nt_{xn_name}", bufs=2))
                pp = ctx.enter_context(tc.tile_pool(name=f"np_{xn_name}",
                                                    bufs=1, space="PSUM"))
                var_ps = [pp.tile([1, BLK], F32, name=f"var{n}",
                                  tag=f"var{n}") for n in range(4)]
                reread = src_hid if dst_hid is None else dst_hid
                for k in range(KT_H):
                    ht = tp.tile([128, L], F32, name="ht", tag="ht")
                    nc.sync.dma_start(out=ht,
                                      in_=src_hid[k * 128:(k + 1) * 128, :])
                    if src_ar is not None:
                        at = tp.tile([128, L], BF, name="at", tag="at")
                        nc.sync.dma_start(
                            out=at, in_=src_ar[k * 128:(k + 1) * 128, :])
                        nc.vector.tensor_add(ht, ht, at)
                    hn = ht
                    if final_out is not None:
                        nc.sync.dma_start(
                            out=final_out[k * 128:(k + 1) * 128, :], in_=hn)
                        continue
                    if dst_hid is not None:
                        nc.sync.dma_start(
                            out=dst_hid[k * 128:(k + 1) * 128, :], in_=hn)
                    sq = tp.tile([128, L], BF, name="sq", tag="sq")
                    nc.scalar.activation(sq, hn, AF.Square)
                    for n in range(4):
                        nc.tensor.matmul(
                            var_ps[n], ones_bf, sq[:, n * BLK:(n + 1) * BLK],
                            start=(k == 0), stop=(k == KT_H - 1))
                if final_out is not None:
                    return None
                srow = tp.tile([1, L], F32, name="srow", tag="srow", bufs=1)
                for n in range(4):
                    nc.scalar.activation(
                        srow[:, n * BLK:(n + 1) * BLK], var_ps[n], AF.Sqrt,
                        scale=1.0 / H, bias=eps_t[0:1, :])
                nc.vector.reciprocal(srow, srow)
                sbc = tp.tile([128, L], F32, name="sbc", tag="sbc", bufs=1)
                nc.gpsimd.partition_broadcast(sbc, srow)
                xn = []
                for k in range(KT_H):
                    ht2 = tp.tile([128, L], F32, name="ht2", tag="ht")
                    nc.sync.dma_start(out=ht2,
                                      in_=reread[k * 128:(k + 1) * 128, :])
                    xk = xn_pool.tile([128, L], BF, name=f"{xn_name}{k}",
                                      tag=f"{xn_name}{k}")
                    nc.vector.tensor_mul(xk, ht2, sbc)
                    xn.append(xk)
                return xn

        # ================= phase 1: input norm =================
        xn1_cm, xn1_pool = mk_pool("xn1_pool", 1)
        xn1 = norm_pass(h0T, None, None, xn1_pool, "xn1")
        xv1, xl1 = strip_mask(xn1_pool, "x1", xn1, KT_H)
        if debug:
            for k in range(KT_H):
                nc.sync.dma_start(
                    out=dbg["dbg_xn1"][k * 128:(k + 1) * 128, :], in_=xn1[k])

        rope_cm, rope = mk_pool("rope", 1)
        rq_c = rope.tile([128, L], BF, name="rq_c")
        nc.sync.dma_start(out=rq_c, in_=ropeq_c[:])
        rq_s = rope.tile([128, L], BF, name="rq_s")
        nc.sync.dma_start(out=rq_s, in_=ropeq_s[:])
        rk_c = rope.tile([128, L], BF, name="rk_c")
        nc.sync.dma_start(out=rk_c, in_=rope_c[:])
        rk_s = rope.tile([128, L], BF, name="rk_s")
        nc.sync.dma_start(out=rk_s, in_=rope_s[:])

        qk_cm, qk_pool = mk_pool("qk_pool", 1, side="right")
        qhat = [qk_pool.tile([128, L], BF, name=f"qhat{h}", tag=f"qhat{h}")
                for h in range(4)]
        khat = [qk_pool.tile([128, L], BF, name=f"khat{h}", tag=f"khat{h}")
                for h in range(4)]
        vt = [qk_pool.tile([128, KT_H, 128], BF, name=f"vt{h}", tag=f"vt{h}")
              for h in range(4)]

        # ================= phase 2: QKV =================
        with ExitStack() as ctx:
            wp = ctx.enter_context(tc.tile_pool(name="qkv_w", bufs=2))
            pp = ctx.enter_context(tc.tile_pool(name="qkv_ps", bufs=6,
                                                space="PSUM"))
            ep = ctx.enter_context(tc.tile_pool(name="qkv_e", bufs=2))
            for m in range(12):
                wv_s = wp.tile([128, KT_H, 128], BF, name="wv_s", tag="wv_s")
                nc.sync.dma_start(out=wv_s,
                                  in_=wqkv["v"][:, :, m * 128:(m + 1) * 128])
                wl_s = wp.tile([128, KT_H, 128], BF, name="wl_s", tag="wl_s")
                nc.sync.dma_start(out=wl_s,
                                  in_=wqkv["l"][:, :, m * 128:(m + 1) * 128])
                ps = [pp.tile([128, BLK], F32, name="qkvps", tag="qkvps")
                      for _ in range(NBLK)]
                for k in range(KT_H):
                    expert_mms(ps, wv_s[:, k, :], wl_s[:, k, :],
                               xn1[k], xv1[k], xl1[k], k, KT_H - 1)
                if m < 8:   # q or k head -> rope
                    h = m % 4
                    dst = qhat[h] if m < 4 else khat[h]
                    tab_c = rq_c if m < 4 else rk_c
                    tab_s = rq_s if m < 4 else rk_s
                    raw = ep.tile([128, L], BF, name="rawqk", tag="rawqk")
                    for b in range(NBLK):
                        nc.scalar.activation(
                            raw[:, b * BLK:(b + 1) * BLK], ps[b], AF.Copy)
                    shf = ep.tile([128, L], BF, name="shf", tag="shf")
                    nc.sync.dma_start(out=shf[0:64, :], in_=raw[64:128, :])
                    nc.sync.dma_start(out=shf[64:128, :], in_=raw[0:64, :])
                    t1 = ep.tile([128, L], BF, name="ropet1", tag="ropet1")
                    nc.vector.tensor_mul(t1, raw, tab_c)
                    t2 = ep.tile([128, L], BF, name="ropet2", tag="ropet2")
                    nc.vector.tensor_mul(t2, shf, tab_s)
                    nc.vector.tensor_add(dst, t1, t2)
                else:       # v head -> copy + per-ktile transpose
                    h = m - 8
                    vh = ep.tile([128, L], BF, name="vh", tag="vh")
                    for b in range(NBLK):
                        nc.scalar.activation(
                            vh[:, b * BLK:(b + 1) * BLK], ps[b], AF.Copy)
                    with tc.tile_pool(name="qkv_tp", bufs=2,
                                      space="PSUM") as tpp:
                        for kt in range(KT_H):
                            tps = tpp.tile([128, 128], BF, name="tps",
                                           tag="tps")
                            nc.tensor.transpose(
                                tps, vh[:, kt * 128:(kt + 1) * 128], ident)
                            nc.scalar.activation(vt[h][:, kt, :], tps,
                                                 AF.Copy)
        close(rope_cm)
        close(xn1_cm)
        if debug:
            for h in range(4):
                nc.sync.dma_start(
                    out=dbg["dbg_qhat"][h * 128:(h + 1) * 128, :], in_=qhat[h])
                nc.sync.dma_start(
                    out=dbg["dbg_khat"][h * 128:(h + 1) * 128, :], in_=khat[h])

        # ================= phase 3: self attention =================
        ctx_cm, ctx_pool = mk_pool("ctx_pool", 1)
        dm_cm, dmp = mk_pool("dmsk", 1)
        dm = dmp.tile([128, 16, BLK], BF, name="dm")
        nc.sync.dma_start(out=dm, in_=dmask[:])
        ctxh = [ctx_pool.tile([128, L], BF, name=f"ctx{h}", tag=f"ctx{h}")
                for h in range(4)]
        with ExitStack() as ctx:
            sp = ctx.enter_context(tc.tile_pool(name="att_s", bufs=3,
                                                space="PSUM"))
            up = ctx.enter_context(tc.tile_pool(name="att_u", bufs=2,
                                                space="PSUM"))
            dp = ctx.enter_context(tc.tile_pool(name="att_d", bufs=2,
                                                space="PSUM"))
            ep = ctx.enter_context(tc.tile_pool(name="att_e", bufs=6))
            for h in range(4):
                for qb in range(NBLK):
                    u_ps = up.tile([128, BLK], F32, name="u_ps", tag="u_ps")
                    d_ps = dp.tile([1, BLK], F32, name="d_ps", tag="d_ps")
                    nkt = qb * 4 + 4
                    for kt in range(nkt):
                        s_ps = sp.tile([128, BLK], F32, name="s_ps",
                                       tag="s_ps")
                        nc.tensor.matmul(
                            s_ps, khat[h][:, kt * 128:(kt + 1) * 128],
                            qhat[h][:, qb * BLK:(qb + 1) * BLK])
                        e_t = ep.tile([128, BLK], BF, name="e_t", tag="e_t")
                        nc.scalar.activation(e_t, s_ps, AF.Exp)
                        if kt // 4 == qb:
                            nc.vector.tensor_mul(
                                e_t, e_t, dm[:, qb * 4 + (kt % 4), :])
                        nc.tensor.matmul(u_ps, vt[h][:, kt, :], e_t,
                                         start=(kt == 0), stop=(kt == nkt - 1))
                        nc.tensor.matmul(d_ps, ones_bf, e_t,
                                         start=(kt == 0), stop=(kt == nkt - 1))
                    rc = ep.tile([1, BLK], F32, name="rc", tag="rc")
                    nc.vector.reciprocal(rc, d_ps)
                    rb = ep.tile([128, BLK], F32, name="rb", tag="rb")
                    nc.gpsimd.partition_broadcast(rb, rc)
                    nc.vector.tensor_mul(
                        ctxh[h][:, qb * BLK:(qb + 1) * BLK], u_ps, rb)
        close(dm_cm)
        close(qk_cm)
        if debug:
            for h in range(4):
                nc.sync.dma_start(
                    out=dbg["dbg_ctx"][h * 128:(h + 1) * 128, :], in_=ctxh[h])

        ctxv, ctxl = strip_mask(ctx_pool, "cx", ctxh, NBLK)

        # ================= phase 4: dense + AR0 =================
        with ExitStack() as ctx:
            wp = ctx.enter_context(tc.tile_pool(name="dns_w", bufs=1))
            pp = ctx.enter_context(tc.tile_pool(name="dns_ps", bufs=8,
                                                space="PSUM"))
            ep = ctx.enter_context(tc.tile_pool(name="dns_e", bufs=3))
            wv_s = wp.tile([128, 4, H], BF, name="wdv")
            nc.sync.dma_start(out=wv_s, in_=wdense["v"][:])
            wl_s = wp.tile([128, 4, H], BF, name="wdl")
            nc.sync.dma_start(out=wl_s, in_=wdense["l"][:])
            for m in range(KT_H):
                ps = [pp.tile([128, BLK], F32, name="dnsps", tag="dnsps")
                      for _ in range(NBLK)]
                for k in range(NBLK):
                    expert_mms(ps, wv_s[:, k, m * 128:(m + 1) * 128],
                               wl_s[:, k, m * 128:(m + 1) * 128],
                               ctxh[k], ctxv[k], ctxl[k], k, NBLK - 1)
                st = ep.tile([128, L], BF, name="dnsst", tag="dnsst")
                for b in range(NBLK):
                    nc.scalar.activation(st[:, b * BLK:(b + 1) * BLK],
                                         ps[b], AF.Copy)
                nc.sync.dma_start(out=ar_in[0][m * 128:(m + 1) * 128, :],
                                  in_=st)
                if m % 4 == 3:
                    ar_chunk(0, m // 4)
        close(ctx_cm)

        # ================= phase 5: residual1 + norm2 (ci) =================
        ci_cm, ci_pool = mk_pool("ci_pool", 1, side="right")
        ci = norm_pass(h0T, ar_out[0], hid1, ci_pool, "ci")

        # ================= phase 6: cross attention =================
        crs_cm, crs = mk_pool("crs_a", 1)
        with ExitStack() as ctx:
            wp = ctx.enter_context(tc.tile_pool(name="crs_w", bufs=1))
            ep = ctx.enter_context(tc.tile_pool(name="crs_e", bufs=3))

            ckv_cm = tc.tile_pool(name="ckv_tmp", bufs=1)
            ckvp = ckv_cm.__enter__()
            et = []
            for k in range(8):
                etk = ckvp.tile([128, LE], BF, name=f"et{k}", tag=f"et{k}")
                nc.sync.dma_start(out=etk, in_=eT[k * 128:(k + 1) * 128, :])
                et.append(etk)
            wkv_s = ckvp.tile([128, 8, 512], BF, name="wkv_s")
            nc.sync.dma_start(out=wkv_s, in_=wckv[:])
            ckh = [crs.tile([128, LE], BF, name=f"ckh{t}", tag=f"ckh{t}")
                   for t in range(2)]
            cvh = [ckvp.tile([128, LE], BF, name=f"cvh{t}", tag=f"cvh{t}")
                   for t in range(2)]
            pp_ckv_cm = tc.tile_pool(name="ps_ckv", bufs=4, space="PSUM")
            pp = pp_ckv_cm.__enter__()
            for m in range(4):  # rows: ck tile0, ck tile1, cv tile0, cv tile1
                dst = ckh[m] if m < 2 else cvh[m - 2]
                ps = [pp.tile([128, BLK], F32, name="ckvps", tag="ckvps")
                      for _ in range(2)]
                for k in range(8):
                    for bb in range(2):
                        nc.tensor.matmul(
                            ps[bb], wkv_s[:, k, m * 128:(m + 1) * 128],
                            et[k][:, bb * BLK:(bb + 1) * BLK],
                            start=(k == 0), stop=(k == 7))
                for bb in range(2):
                    nc.scalar.activation(dst[:, bb * BLK:(bb + 1) * BLK],
                                         ps[bb], AF.Copy)
            pp_ckv_cm.__exit__(None, None, None)
            vc = [crs.tile([128, 8, CHD], BF, name=f"vc{h}", tag=f"vc{h}")
                  for h in range(4)]
            pp_tp_cm = tc.tile_pool(name="ps_ctp", bufs=2, space="PSUM")
            tpp = pp_tp_cm.__enter__()
            for h in range(4):
                t, base = h // 2, (h % 2) * 64
                for kt in range(8):
                    tps = tpp.tile([128, CHD], BF, name="ctps", tag="ctps")
                    nc.tensor.transpose(
                        tps, cvh[t][base:base + 64, kt * 128:(kt + 1) * 128],
                        ident[base:base + 64, base:base + 64])
                    nc.scalar.activation(vc[h][:, kt, :], tps, AF.Copy)
            if debug:
                for t in range(2):
                    nc.sync.dma_start(
                        out=dbg["dbg_ck"][t * 128:(t + 1) * 128, :],
                        in_=ckh[t])

            ckv_cm.__exit__(None, None, None)
            wq_s = wp.tile([128, KT_H, 256], BF, name="wq_s")
            nc.sync.dma_start(out=wq_s, in_=wcq[:])
            pp_tp_cm.__exit__(None, None, None)
            cqh = [crs.tile([128, L], BF, name=f"cqh{t}", tag=f"cqh{t}")
                   for t in range(2)]
            pp_cq_cm = tc.tile_pool(name="ps_cq", bufs=4, space="PSUM")
            pp = pp_cq_cm.__enter__()
            for m in range(2):
                ps = [pp.tile([128, BLK], F32, name="cqps", tag="cqps")
                      for _ in range(NBLK)]
                for k in range(KT_H):
                    for bb in range(NBLK):
                        nc.tensor.matmul(
                            ps[bb], wq_s[:, k, m * 128:(m + 1) * 128],
                            ci[k][:, bb * BLK:(bb + 1) * BLK],
                            start=(k == 0), stop=(k == KT_H - 1))
                for bb in range(NBLK):
                    nc.scalar.activation(cqh[m][:, bb * BLK:(bb + 1) * BLK],
                                         ps[bb], AF.Copy)
            close(ci_cm)
            if debug:
                for t in range(2):
                    nc.sync.dma_start(
                        out=dbg["dbg_cq"][t * 128:(t + 1) * 128, :],
                        in_=cqh[t])

            pp_cq_cm.__exit__(None, None, None)
            cctx = [crs.tile([64, L], BF, name=f"cctx{h}", tag=f"cctx{h}")
                    for h in range(4)]
            pp_fl_cm = tc.tile_pool(name="ps_cfl", bufs=2, space="PSUM")
            pp = pp_fl_cm.__enter__()
            for h in range(4):
                t, base = h // 2, (h % 2) * 64
                for qb in range(NBLK):
                    u_ps = pp.tile([64, BLK], F32, name="cu_ps", tag="cu_ps")
                    d_ps = pp.tile([1, BLK], F32, name="cd_ps", tag="cd_ps")
                    for kt in range(8):
                        s_ps = pp.tile([128, BLK], F32, name="cs_ps",
                                       tag="cs_ps", bufs=3)
                        nc.tensor.matmul(
                            s_ps,
                            ckh[t][base:base + 64, kt * 128:(kt + 1) * 128],
                            cqh[t][base:base + 64, qb * BLK:(qb + 1) * BLK])
                        e_t = ep.tile([128, BLK], BF, name="ce_t", tag="ce_t")
                        nc.scalar.activation(e_t, s_ps, AF.Exp)
                        nc.tensor.matmul(u_ps, vc[h][:, kt, :], e_t,
                                         start=(kt == 0), stop=(kt == 7))
                        nc.tensor.matmul(d_ps, ones_bf, e_t,
                                         start=(kt == 0), stop=(kt == 7))
                    rc = ep.tile([1, BLK], F32, name="crc", tag="crc")
                    nc.vector.reciprocal(rc, d_ps)
                    rb = ep.tile([64, BLK], F32, name="crb", tag="crb")
                    nc.gpsimd.partition_broadcast(rb, rc)
                    nc.vector.tensor_mul(
                        cctx[h][:, qb * BLK:(qb + 1) * BLK], u_ps, rb)
            cpk = [crs.tile([128, L], BF, name=f"cpk{t}", tag=f"cpk{t}")
                   for t in range(2)]
            for t in range(2):
                nc.sync.dma_start(out=cpk[t][0:64, :], in_=cctx[2 * t])
                nc.sync.dma_start(out=cpk[t][64:128, :], in_=cctx[2 * t + 1])
            if debug:
                for t in range(2):
                    nc.sync.dma_start(
                        out=dbg["dbg_cctx"][t * 128:(t + 1) * 128, :],
                        in_=cpk[t])

            pp_fl_cm.__exit__(None, None, None)
            wp2 = ctx.enter_context(tc.tile_pool(name="crs_w2", bufs=1))
            wcd_s = wp2.tile([128, 2, H], BF, name="wcd_s")
            nc.sync.dma_start(out=wcd_s, in_=wcd[:])
            pp_cd_cm = tc.tile_pool(name="ps_cd", bufs=8, space="PSUM")
            pp = pp_cd_cm.__enter__()
            for m in range(KT_H):
                ps = [pp.tile([128, BLK], F32, name="cdps", tag="cdps")
                      for _ in range(NBLK)]
                for k in range(2):
                    for bb in range(NBLK):
                        nc.tensor.matmul(
                            ps[bb], wcd_s[:, k, m * 128:(m + 1) * 128],
                            cpk[k][:, bb * BLK:(bb + 1) * BLK],
                            start=(k == 0), stop=(k == 1))
                st = ep.tile([128, L], BF, name="cdst", tag="cdst")
                for bb in range(NBLK):
                    nc.scalar.activation(st[:, bb * BLK:(bb + 1) * BLK],
                                         ps[bb], AF.Copy)
                nc.sync.dma_start(out=ar_in[1][m * 128:(m + 1) * 128, :],
                                  in_=st)
                if m % 4 == 3:
                    ar_chunk(1, m // 4)
            pp_cd_cm.__exit__(None, None, None)
        close(crs_cm)

        # ================= phase 7: residual2 + norm3 (mi) =================
        mi_cm, mi_pool = mk_pool("mi_pool", 1, side="right")
        mi = norm_pass(hid1, ar_out[1], hid2, mi_pool, "mi")
        miv, mil = strip_mask(mi_pool, "m3", mi, KT_H)

        # ================= phase 8: MLP =================
        hm_cm, hm_pool = mk_pool("hm_pool", 1)
        hm = [hm_pool.tile([128, L], BF, name=f"hm{m}", tag=f"hm{m}")
              for m in range(11)]
        with ExitStack() as ctx:
            wp_cm = tc.tile_pool(name="mlp_w", bufs=2)
            wp = wp_cm.__enter__()
            ep_cm = tc.tile_pool(name="mlp_e", bufs=3)
            ep = ep_cm.__enter__()
            gp_cm = tc.tile_pool(name="mlp_gps", bufs=4, space="PSUM")
            gp = gp_cm.__enter__()
            upp_cm = tc.tile_pool(name="mlp_ups", bufs=4, space="PSUM")
            upp = upp_cm.__enter__()
            for m in range(11):
                wgv = wp.tile([128, KT_H, 128], BF, name="wgv", tag="wgv")
                nc.sync.dma_start(out=wgv,
                                  in_=wg["v"][:, :, m * 128:(m + 1) * 128])
                wgl = wp.tile([128, KT_H, 128], BF, name="wgl", tag="wgl")
                nc.sync.dma_start(out=wgl,
                                  in_=wg["l"][:, :, m * 128:(m + 1) * 128])
                wuv = wp.tile([128, KT_H, 128], BF, name="wuv", tag="wuv")
                nc.sync.dma_start(out=wuv,
                                  in_=wu["v"][:, :, m * 128:(m + 1) * 128])
                wul = wp.tile([128, KT_H, 128], BF, name="wul", tag="wul")
                nc.sync.dma_start(out=wul,
                                  in_=wu["l"][:, :, m * 128:(m + 1) * 128])
                gps = [gp.tile([128, BLK], F32, name="gps", tag="gps")
                       for _ in range(NBLK)]
                ups = [upp.tile([128, BLK], F32, name="ups", tag="ups")
                       for _ in range(NBLK)]
                for k in range(KT_H):
                    expert_mms(gps, wgv[:, k, :], wgl[:, k, :],
                               mi[k], miv[k], mil[k], k, KT_H - 1)
                    expert_mms(ups, wuv[:, k, :], wul[:, k, :],
                               mi[k], miv[k], mil[k], k, KT_H - 1)
                for b in range(NBLK):
                    sg = ep.tile([128, BLK], BF, name="sg", tag="sg")
                    nc.scalar.activation(sg, gps[b], AF.Silu)
                    nc.vector.tensor_mul(
                        hm[m][:, b * BLK:(b + 1) * BLK], sg, ups[b])
            upp_cm.__exit__(None, None, None)
            gp_cm.__exit__(None, None, None)
            ep_cm.__exit__(None, None, None)
            wp_cm.__exit__(None, None, None)
            close(mi_cm)
            hmv, hml = strip_mask(hm_pool, "hm", hm, 11)
            wp = ctx.enter_context(tc.tile_pool(name="mlp_w2", bufs=2))
            ep = ctx.enter_context(tc.tile_pool(name="mlp_e2", bufs=2))
            if debug:
                for m in range(11):
                    nc.sync.dma_start(
                        out=dbg["dbg_hmlp"][m * 128:(m + 1) * 128, :],
                        in_=hm[m])
            dw_cm = tc.tile_pool(name="mlp_dps", bufs=8, space="PSUM")
            dwp = dw_cm.__enter__()
            for mc in range(8):
                wdv = wp.tile([128, 11, 256], BF, name="wdnv", tag="wdnv")
                nc.sync.dma_start(out=wdv,
                                  in_=wd["v"][:, :, mc * 256:(mc + 1) * 256])
                wdl = wp.tile([128, 11, 256], BF, name="wdnl", tag="wdnl")
                nc.sync.dma_start(out=wdl,
                                  in_=wd["l"][:, :, mc * 256:(mc + 1) * 256])
                for mloc in range(2):
                    m = mc * 2 + mloc
                    ps = [dwp.tile([128, BLK], F32, name="dwps", tag="dwps")
                          for _ in range(NBLK)]
                    for k in range(11):
                        expert_mms(ps, wdv[:, k, mloc * 128:(mloc + 1) * 128],
                                   wdl[:, k, mloc * 128:(mloc + 1) * 128],
                                   hm[k], hmv[k], hml[k], k, 10)
                    st = ep.tile([128, L], BF, name="dwst", tag="dwst")
                    for b in range(NBLK):
                        nc.scalar.activation(st[:, b * BLK:(b + 1) * BLK],
                                             ps[b], AF.Copy)
                    nc.sync.dma_start(out=ar_in[2][m * 128:(m + 1) * 128, :],
                                      in_=st)
                    if m % 4 == 3:
                        ar_chunk(2, m // 4)
            dw_cm.__exit__(None, None, None)
        close(hm_cm)

        # ================= phase 9: residual3 -> out =================
        norm_pass(hid2, ar_out[2], None, None, "fin", final_out=outT)

    nc.compile()
    return nc


def _get_program(meta, debug):
    key = (tuple(meta["sA"]), tuple(meta["sB"]), debug)
    if key not in _CACHE:
        _CACHE[key] = build_program(meta, debug)
    return _CACHE[key]


def run(inputs, debug=False, trace=False):
    from concourse.bass_utils import run_bass_kernel_spmd
    in_maps, meta = host_prep(inputs)
    nc = _get_program(meta, debug)
    res = run_bass_kernel_spmd(nc, in_maps, list(range(N_CORES)),
                               trace=trace)
    outs = []
    for b in range(B):
        perm = meta["perms"][b]
        oT = res.results[b * TP]["outT"]          # [H, L] f32, permuted
        ob = np.empty((L, H), np.float32)
        ob[perm] = np.ascontiguousarray(oT.T)
        outs.append(ob)
    out = np.stack(outs).astype(np.float32)
    return out, res, meta


def kernel(**inputs) -> np.ndarray:
    out, _, _ = run(inputs)
    return out
